# revision 3
# baseline (speedup 1.0000x reference)
"""Trainium2 Bass kernel for nn_CognitiveRouter (hierarchical MoE routing).

Computation (see reference):
    module_logits = h @ Wm.T      (T,4)
    expert_logits = h @ We.T      (T,16)
    module_probs  = softmax(module_logits)
    expert_probs  = softmax(expert_logits.reshape(T,4,4), axis=-1)
    combined      = (module_probs[:,:,None]*expert_probs).reshape(T,16)
    topw, topi    = top_k(combined, 4);  topw /= (sum(topw)+1e-8)

Strategy:
  - Data-parallel: shard T=32768 across 8 NeuronCores (4096 tokens each).
  - Host prep: W = [Wm;We] (20,1536) and h are split into bf16 hi/lo pairs
    (x = hi + lo captures ~2^-18 relative accuracy, fp32-class logits) and
    transposed so the contraction dim D lands on SBUF partitions.
    (h + lo)@(Whi + Wlo) is computed with 2 matmul passes per k-tile into
    one PSUM accumulator of 40 columns [hi·Whi + lo·Whi | hi·Wlo + lo·Wlo],
    then logits = psum[:, :20] + psum[:, 20:40]  (all 4 cross terms).
  - Per core: 4 quarters of 1024 tokens; each quarter loads [128,12,1024]
    bf16 hi/lo slabs (3 MB DMAs), runs 8 token-tiles x 24 accumulating
    matmuls (stationary = h tile, moving = W 40 cols), then a packed
    softmax + top-k epilogue on [128, 8, 20] tiles. Top-4 via vector.max
    (top-8 sorted desc) + max_index (ties -> ascending distinct indices,
    matching jax.lax.top_k).
"""

import sys

if "/opt/trn_rl_repo" not in sys.path:
    sys.path.insert(0, "/opt/trn_rl_repo")

import ml_dtypes
import numpy as np

import concourse.bacc as bacc
import concourse.mybir as mybir
import concourse.tile as tile
from concourse.bass_utils import run_bass_kernel_spmd

N_CORES = 8
T, D = 32768, 1536
TLOC = T // N_CORES          # 4096 tokens per core
NQ = 4                       # quarters per core
QT = TLOC // NQ              # 1024 tokens per quarter
NTT = QT // 128              # 8 token tiles (of 128) per quarter
NKT = D // 128               # 12 k-tiles
NE = 20                      # 4 module + 16 expert logit columns

BF16 = mybir.dt.bfloat16
F32 = mybir.dt.float32
U32 = mybir.dt.uint32
AX = mybir.AxisListType.X
EXP = mybir.ActivationFunctionType.Exp

_CACHE = {}
LAST_RESULT = None  # BassKernelResults of the most recent run (for profiling)


def _build():
    nc = bacc.Bacc(trn_type="TRN2", target_bir_lowering=False, debug=False)

    hiT = nc.dram_tensor("hiT", [D, TLOC], BF16, kind="ExternalInput")
    loT = nc.dram_tensor("loT", [D, TLOC], BF16, kind="ExternalInput")
    wT = nc.dram_tensor("wT", [D, 2 * NE], BF16, kind="ExternalInput")
    o_probs = nc.dram_tensor("o_probs", [TLOC, 16], F32, kind="ExternalOutput")
    o_topw = nc.dram_tensor("o_topw", [TLOC, 4], F32, kind="ExternalOutput")
    o_topi = nc.dram_tensor("o_topi", [TLOC, 4], U32, kind="ExternalOutput")

    # DRAM views with the 128-partition dim innermost on the left
    hiT_v = hiT.ap().rearrange("(k p) t -> p k t", p=128)   # [128, 12, 4096]
    loT_v = loT.ap().rearrange("(k p) t -> p k t", p=128)
    wT_v = wT.ap().rearrange("(k p) e -> p k e", p=128)     # [128, 12, 40]
    # token t_local = q*QT + i*128 + p
    probs_v = o_probs.ap().rearrange("(q i p) e -> q p i e", p=128, i=NTT)
    topw_v = o_topw.ap().rearrange("(q i p) e -> q p i e", p=128, i=NTT)
    topi_v = o_topi.ap().rearrange("(q i p) e -> q p i e", p=128, i=NTT)

    with tile.TileContext(nc) as tc:
        with (
            tc.tile_pool(name="kt", bufs=3) as ktp,
            tc.tile_pool(name="wp", bufs=1) as wp,
            tc.tile_pool(name="ps", bufs=8, space="PSUM") as pp,
            tc.tile_pool(name="ep", bufs=2) as ep,
            tc.tile_pool(name="outp", bufs=3) as outp,
        ):
            w_sb = wp.tile([128, NKT, 2 * NE], BF16)
            nc.sync.dma_start(out=w_sb, in_=wT_v)

            for q in range(NQ):
                hi_sb = ktp.tile([128, NKT, QT], BF16, tag="hi")
                lo_sb = ktp.tile([128, NKT, QT], BF16, tag="lo")
                nc.sync.dma_start(out=hi_sb, in_=hiT_v[:, :, q * QT:(q + 1) * QT])
                nc.sync.dma_start(out=lo_sb, in_=loT_v[:, :, q * QT:(q + 1) * QT])

                # ---- matmuls: per token tile, 24 accumulating matmuls ----
                ps_list = []
                for i in range(NTT):
                    ps = pp.tile([128, 2 * NE], F32)
                    n_mm = 2 * NKT
                    j = 0
                    for k in range(NKT):
                        for src in (hi_sb, lo_sb):
                            nc.tensor.matmul(
                                ps,
                                lhsT=src[:, k, i * 128:(i + 1) * 128],
                                rhs=w_sb[:, k, :],
                                start=(j == 0),
                                stop=(j == n_mm - 1),
                            )
                            j += 1
                    ps_list.append(ps)

                # ---- epilogue: logits -> hierarchical softmax -> top-4 ----
                # PSUM -> SBUF on ScalarE (one PSUM read port; DVE cannot read
                # two PSUM operands), then a single SBUF-only add folds the
                # [hi*Whi+lo*Whi | hi*Wlo+lo*Wlo] halves.
                ps_sb = ep.tile([128, NTT, 2 * NE], F32, tag="ps_sb")
                for i in range(NTT):
                    nc.scalar.activation(
                        ps_sb[:, i, :], ps_list[i], mybir.ActivationFunctionType.Copy
                    )
                logits = ep.tile([128, NTT, NE], F32, tag="logits")
                nc.vector.tensor_add(
                    logits, ps_sb[:, :, 0:NE], ps_sb[:, :, NE:2 * NE]
                )

                lg_m = logits[:, :, 0:4]                                   # [128,8,4]
                lg_e = logits[:, :, 4:NE].rearrange("p g (m e) -> p g m e", e=4)

                mmax = ep.tile([128, NTT], F32, tag="mmax")
                emax = ep.tile([128, NTT, 4], F32, tag="emax")
                nc.vector.reduce_max(mmax, lg_m, axis=AX)
                nc.vector.reduce_max(emax, lg_e, axis=AX)

                expsrc = ep.tile([128, NTT, NE], F32, tag="expsrc")
                nc.vector.tensor_sub(
                    expsrc[:, :, 0:4], lg_m, mmax.to_broadcast([128, NTT, 4])
                )
                nc.vector.tensor_sub(
                    expsrc[:, :, 4:NE].rearrange("p g (m e) -> p g m e", e=4),
                    lg_e,
                    emax.to_broadcast([128, NTT, 4, 4]),
                )
                expv = ep.tile([128, NTT, NE], F32, tag="expv")
                nc.scalar.activation(expv, expsrc, EXP)

                msum = ep.tile([128, NTT], F32, tag="msum")
                esum = ep.tile([128, NTT, 4], F32, tag="esum")
                nc.vector.reduce_sum(msum, expv[:, :, 0:4], axis=AX)
                nc.vector.reduce_sum(
                    esum, expv[:, :, 4:NE].rearrange("p g (m e) -> p g m e", e=4),
                    axis=AX,
                )
                denom = ep.tile([128, NTT, 4], F32, tag="denom")
                nc.vector.tensor_mul(denom, esum, msum.to_broadcast([128, NTT, 4]))
                rden = ep.tile([128, NTT, 4], F32, tag="rden")
                nc.vector.reciprocal(rden, denom)
                coef = ep.tile([128, NTT, 4], F32, tag="coef")
                nc.vector.tensor_mul(coef, expv[:, :, 0:4], rden)

                comb = outp.tile([128, NTT, 16], F32, tag="comb")
                nc.vector.tensor_mul(
                    comb.rearrange("p g (m e) -> p g m e", e=4),
                    expv[:, :, 4:NE].rearrange("p g (m e) -> p g m e", e=4),
                    coef.to_broadcast([128, NTT, 4, 4]),
                )
                nc.scalar.dma_start(out=probs_v[q], in_=comb)

                maxv = ep.tile([128, NTT, 8], F32, tag="maxv")
                idx = outp.tile([128, NTT, 8], U32, tag="idx")
                for i in range(NTT):
                    nc.vector.max(out=maxv[:, i, :], in_=comb[:, i, :])
                    nc.vector.max_index(
                        out=idx[:, i, :], in_max=maxv[:, i, :], in_values=comb[:, i, :]
                    )

                wsum = ep.tile([128, NTT], F32, tag="wsum")
                nc.vector.reduce_sum(wsum, maxv[:, :, 0:4], axis=AX)
                nc.vector.tensor_scalar_add(wsum, wsum, 1e-8)
                rw = ep.tile([128, NTT], F32, tag="rw")
                nc.vector.reciprocal(rw, wsum)
                topw = outp.tile([128, NTT, 4], F32, tag="topw")
                nc.vector.tensor_mul(
                    topw, maxv[:, :, 0:4], rw.to_broadcast([128, NTT, 4])
                )
                nc.scalar.dma_start(out=topw_v[q], in_=topw)
                nc.scalar.dma_start(out=topi_v[q], in_=idx[:, :, 0:4])

    nc.compile()
    return nc


def _get_nc():
    if "nc" not in _CACHE:
        _CACHE["nc"] = _build()
    return _CACHE["nc"]


def _split_bf16(x32):
    """x32 (f32) -> (hi, lo) bf16 with hi + lo ~= x32 (~2^-18 rel)."""
    bf = ml_dtypes.bfloat16
    hi = x32.astype(bf)
    lo = (x32 - hi.astype(np.float32)).astype(bf)
    return hi, lo


def kernel(hidden_states, Wm, We):
    global LAST_RESULT
    nc = _get_nc()

    h = np.asarray(hidden_states, dtype=np.float32)
    W = np.concatenate(
        [np.asarray(Wm, dtype=np.float32), np.asarray(We, dtype=np.float32)], axis=0
    )  # [20, 1536]

    w_hi, w_lo = _split_bf16(W)
    wT = np.ascontiguousarray(
        np.concatenate([w_hi.T, w_lo.T], axis=1)
    )  # [1536, 40] bf16

    h_hi, h_lo = _split_bf16(h)

    in_maps = []
    for c in range(N_CORES):
        sl = slice(c * TLOC, (c + 1) * TLOC)
        in_maps.append(
            {
                "hiT": np.ascontiguousarray(h_hi[sl].T),
                "loT": np.ascontiguousarray(h_lo[sl].T),
                "wT": wT,
            }
        )

    res = run_bass_kernel_spmd(nc, in_maps, core_ids=list(range(N_CORES)))
    LAST_RESULT = res

    probs = np.concatenate([res.results[c]["o_probs"] for c in range(N_CORES)], axis=0)
    topw = np.concatenate([res.results[c]["o_topw"] for c in range(N_CORES)], axis=0)
    topi = np.concatenate(
        [res.results[c]["o_topi"] for c in range(N_CORES)], axis=0
    ).astype(np.int32)
    return probs, topw, topi
